# revision 60
# baseline (speedup 1.0000x reference)
"""Trainium2 Bass kernel for batched TreeCRF message passing.

Reference semantics (per depth layer d):
    x[b,c,w]   = emissions[b,c,layer[w]] + messages[b,c,layer[w]]
    elem[n,b,k,w] = logsumexp_c(x[b,c,w] + transitions[n, layer[w], k, c])
    messages[b,k,n] += sum_w elem[n,b,k,w] * succ[d,w,n]

The successor matrix rows are one-hot, so only the 2x2 transition block of the
single target node per source survives, and the dense scatter is a 100x100
matmul per layer (collisions summed exactly).  Using the identity
    lse(s0, s1) = s1 + softplus(s0 - s1),   softplus(x) = ln(exp(x) + 1)
each scan step runs the minimal cross-engine chain
    V:   t  = dbase_d + P0;  diff = t - P1     (P = incoming messages, psum)
    ACT: sp = ln(exp(diff) + 1)                (exp + ln, one table set)
    PE:  P' = S_d^T s1 + S_d^T sp              (two accumulating matmuls)
with V also computing s1 = base1_d + P1, archiving each finished psum layer
to SBUF, and precomputing base_d = emissions + transitions / dbase_d =
base_d[c0] - base_d[c1] for upcoming layers in its idle windows.  ~1.51us per
step, limited by semaphore hops and the exp/ln pair (no softplus table in
this toolchain).

IMPORTANT: the DVE does not interlock same-engine read-after-write; every
dependent V op is separated by >=2 intervening instructions (~140ns+ margin).
A thinner 1-op gap (59ns) failed intermittently (~1 in 6 runs, rel err 3e-2)
under timing jitter -- do not re-tighten these gaps for the ~80ns/step it
would save.

Precision: psum accumulates fp32; sp (<= ln 2) and step 1's s1 operand
travel as fp16 so those matmuls are single-pass; every later s1 (|s1| up to
~110) uses fp32 weights/rhs, hidden under the exp/ln window.  Successor
entries are 0/1, exact in both dtypes.

DMAs: completion latency is ~2.7us and transfers on one queue land ~2.2us
apart, so each queue carries exactly one early-critical transfer (sync:
ett layers 0-1; scalar: sbk16 layers 0-1; gpsimd feeds the bulk in deadline
order), and compute starts ~9.9us in, right off the first chunk.

Host side only gathers/reorders inputs (no arithmetic); batch is sharded
8-way across cores; transitions/successor blocks replicated; no cross-core
communication.  Falls back to a faithful numpy implementation if the one-hot
structure does not hold.
"""

import numpy as np

BATCH, C, N_LABELS = 64, 2, 1000
DEPTH, WIDTH = 10, 100
N_CORES = 8
B = BATCH // N_CORES  # batch elements per core
KB = C * B            # 16
D1 = DEPTH - 1        # 9 scan steps / active layers

_BASS_CACHE = {}


def _no_barrier_block(nc, bass):
    """Like nc.Block() but skips the exit drain + all-engine barrier (~6us of
    event-semaphore ping-pong after the output DMA has already been issued)."""
    from contextlib import contextmanager

    class _NBBlock(bass.BassBlock):
        def __exit__(self, exc_type, exc_val, exc_tb):
            if exc_type is None:
                for engine, last_body in self.last_body.items():
                    with self.bass.body(
                        last_body,
                        parent=self.bass.cur_bb,
                        allow_existing_parent=True,
                    ):
                        engine.br(self.end_bb)
                self.bass.switch_bb(self.end_bb)

    @contextmanager
    def _ctx():
        assert nc.cur_block is None
        with _NBBlock(nc, f"block_{nc.next_id()}") as blk:
            nc.cur_block = blk
            yield blk
        nc.cur_block = None

    return _ctx()


def _build_bass():
    import concourse.bass as bass
    import concourse.mybir as mybir

    W = WIDTH
    F32 = mybir.dt.float32
    F16 = mybir.dt.float16
    EXP = mybir.ActivationFunctionType.Exp
    LN = mybir.ActivationFunctionType.Ln

    nc = bass.Bass()
    # ett[w, d, 0:32]  = emissions dup'd over k, layout (c, k, b)
    # ett[w, d, 32:64] = gathered transitions dup'd over b, layout (c, k, b)
    ett = nc.declare_dram_parameter("ett", [W, D1, 64], F32, isOutput=False)
    # successor blocks: fp16 for the sp matmuls (values <= ln 2) and the
    # step-1 s1 matmul (|s1| <= 12, and fp32 weights cannot arrive that
    # early); exact fp32 weights for every other s1 matmul, split into two
    # transfers by DMA deadline (layers 2-3 ~step 2, layers 4-8 ~step 4)
    sbk32b = nc.declare_dram_parameter("sbk32b", [W, 2, W], F32, isOutput=False)
    sbk32 = nc.declare_dram_parameter("sbk32", [W, D1 - 4, W], F32, isOutput=False)
    sbk16 = nc.declare_dram_parameter("sbk16", [W, D1, W], F16, isOutput=False)
    # out[w, j, (c, b)] = messages into layer j+1 (layer 0 receives nothing)
    out = nc.declare_dram_parameter("out", [W, D1, KB], F32, isOutput=True)

    from contextlib import ExitStack

    with ExitStack() as _es:
        ett_s = _es.enter_context(nc.sbuf_tensor("ett_s", [W, D1, 64], F32))
        sbk32b_s = _es.enter_context(nc.sbuf_tensor("sbk32b_s", [W, 2, W], F32))
        sbk32_s = _es.enter_context(nc.sbuf_tensor("sbk32_s", [W, D1 - 4, W], F32))
        sbk16_s = _es.enter_context(nc.sbuf_tensor("sbk16_s", [W, D1, W], F16))
        base_s = _es.enter_context(nc.sbuf_tensor("base_s", [W, D1, 32], F32))
        dbase_s = _es.enter_context(nc.sbuf_tensor("dbase_s", [W, D1, KB], F32))
        diff_s = _es.enter_context(nc.sbuf_tensor("diff_s", [W, 2, KB], F32))
        dpt_s = _es.enter_context(nc.sbuf_tensor("dpt_s", [W, KB], F32))
        sp_s = _es.enter_context(nc.sbuf_tensor("sp_s", [W, 2, KB], F16))
        esp_s = _es.enter_context(nc.sbuf_tensor("esp_s", [W, KB], F32))
        s1f_s = _es.enter_context(nc.sbuf_tensor("s1f_s", [W, 2, KB], F16))
        s1w_s = _es.enter_context(nc.sbuf_tensor("s1w_s", [W, 2, KB], F32))
        s10_s = _es.enter_context(nc.sbuf_tensor("s10_s", [W, KB], F16))
        msg_s = _es.enter_context(nc.sbuf_tensor("msg_s", [W, D1, KB], F32))
        scr_s = _es.enter_context(nc.sbuf_tensor("scr_s", [1, 2], F32))
        ptA = _es.enter_context(nc.psum_tensor("ptA", [128, KB], F32))
        ptB = _es.enter_context(nc.psum_tensor("ptB", [128, KB], F32))
        q_ett1 = _es.enter_context(nc.semaphore("q_ett1"))
        q_ett2 = _es.enter_context(nc.semaphore("q_ett2"))
        q_sbk16a = _es.enter_context(nc.semaphore("q_sbk16a"))
        q_sbk16b = _es.enter_context(nc.semaphore("q_sbk16b"))
        q_sbk32b = _es.enter_context(nc.semaphore("q_sbk32b"))
        q_sbk32 = _es.enter_context(nc.semaphore("q_sbk32"))
        q_out = _es.enter_context(nc.semaphore("q_out"))
        v2s = _es.enter_context(nc.semaphore("v2s"))
        v2t = _es.enter_context(nc.semaphore("v2t"))
        a2t = _es.enter_context(nc.semaphore("a2t"))
        p2t = _es.enter_context(nc.semaphore("p2t"))
        p2v = _es.enter_context(nc.semaphore("p2v"))
        pdone = _es.enter_context(nc.semaphore("pdone"))
        block = _es.enter_context(_no_barrier_block(nc, bass))
        pts = [ptA, ptB]

        def p_lo(d):
            # c=0 half of the step-d psum, k-broadcast to (k, b)
            p = pts[d % 2][:W, :B]
            return p[:, None, :].to_broadcast([W, C, B])

        def p_hi(d):
            p = pts[d % 2][:W, B:]
            return p[:, None, :].to_broadcast([W, C, B])

        def kb(ap):
            return ap.rearrange("w (k b) -> w k b", b=B)

        def base(vector, d):
            # base_d[w, (c,k,b)] = em_dup + tt_dup   (one flat 32-wide add)
            vector.tensor_add(base_s[:, d], ett_s[:, d, :32], ett_s[:, d, 32:])

        def dbase(vector, d):
            return vector.tensor_sub(
                dbase_s[:, d], base_s[:, d, :KB], base_s[:, d, KB:]
            )

        @block.scalar
        def _(scalar):
            # one early-critical DMA, then the dummy activation pulls the
            # exp/ln table load (~1.5us) off the critical path
            scalar.dma_start(sbk16_s[:, :2], sbk16[:, :2]).then_inc(q_sbk16a, 16)
            scalar.activation(scr_s[:1, :1], scr_s[:1, 1:], EXP)
            scalar.drain()
            for d in range(D1):
                scalar.wait_ge(v2s, d + 1)
                src = dbase_s[:, 0] if d == 0 else diff_s[:, d % 2]
                # softplus(diff) = ln(exp(diff) + 1); exp/ln share one ACT
                # table so there is no per-step table swap
                scalar.activation(esp_s[:], src, EXP)
                scalar.nop(cycle_cnt=24)
                scalar.activation(sp_s[:, d % 2], esp_s[:], LN, bias=1.0).then_inc(
                    a2t, 1
                )

        @block.sync
        def _(sync):
            # layers 0-1 of ett gate the whole pipeline: fastest queue, first.
            # DMAs on one queue complete ~2.2us apart, so each queue carries
            # exactly one early-critical transfer.
            sync.dma_start(ett_s[:, :2], ett[:, :2]).then_inc(q_ett1, 16)
            sync.dma_start(sbk32b_s[:], sbk32b[:]).then_inc(q_sbk32b, 16)
            sync.wait_ge(pdone, 1)
            sync.dma_start(out[:], msg_s[:]).then_inc(q_out, 16)

        @block.gpsimd
        def _(gpsimd):
            # Pool cannot touch PSUM; it just feeds the bulk transfers in
            # deadline order (ett2 ~step1, sbk16b ~step2, sbk32 ~step4)
            gpsimd.dma_start(ett_s[:, 2:], ett[:, 2:]).then_inc(q_ett2, 16)
            gpsimd.dma_start(sbk16_s[:, 2:], sbk16[:, 2:]).then_inc(q_sbk16b, 16)
            gpsimd.dma_start(sbk32_s[:], sbk32[:]).then_inc(q_sbk32, 16)

        @block.vector
        def _(vector):
            # NOTE: DVE does not interlock RAW within the engine; dependent
            # ops are separated by an independent op (issue spacing ~95ns,
            # op duration ~190ns).
            vector.wait_ge(q_ett1, 16)
            base(vector, 0)
            base(vector, 1)
            # independent spacer widens every pre-loop RAW margin
            vector.tensor_copy(dpt_s[:], ett_s[:, 0, :KB])
            dbase(vector, 0).then_inc(v2s, 1)
            vector.tensor_copy(s10_s[:], base_s[:, 0, KB:]).then_inc(v2t, 1)
            dbase(vector, 1)
            for d in range(1, D1):
                vector.wait_ge(p2v, d)
                # diff = dbase + P0 - P1 in two ops (an instruction may read
                # at most one PSUM operand)
                vector.tensor_add(
                    kb(dpt_s[:]), kb(dbase_s[:, d]), p_lo(d - 1)
                )
                # s1_d = base_d[c=1] + P1 (k-broadcast); doubles as the RAW
                # pipe gap before diff reads the partial sum
                s1dst = s1f_s if d == 1 else s1w_s
                vector.tensor_add(
                    kb(s1dst[:, d % 2]), kb(base_s[:, d, KB:]), p_hi(d - 1)
                ).then_inc(p2t, 1)
                # archive the finished message layer (psum -> sbuf); also
                # widens the RAW margin before diff reads the partial sum
                vector.tensor_copy(msg_s[:, d - 1], pts[(d - 1) % 2][:W, :])
                vector.tensor_sub(
                    kb(diff_s[:, d % 2]), kb(dpt_s[:]), p_hi(d - 1)
                ).then_inc(v2s, 1)
                # slack-time precompute for upcoming layers (idle window
                # until the next matmul lands; never on the critical path)
                if d == 1:
                    vector.wait_ge(q_ett2, 16)
                    base(vector, 2)
                    base(vector, 3)
                    # spacer: dbase_2 must read base_2 with >=2 ops between
                    vector.tensor_copy(dpt_s[:], ett_s[:, 2, :KB])
                    dbase(vector, 2)
                elif d == 2:
                    base(vector, 4)
                    dbase(vector, 3)
                elif 3 <= d <= 6:
                    base(vector, d + 2)
                    dbase(vector, d + 1)
                elif d == 7:
                    dbase(vector, 8)
            vector.wait_ge(p2v, D1)
            vector.tensor_copy(msg_s[:, D1 - 1], pts[(D1 - 1) % 2][:W, :]).then_inc(
                pdone, 1
            )

        @block.tensor
        def _(tensor):
            tensor.wait_ge(q_sbk16a, 16)
            tensor.wait_ge(v2t, 1)
            tensor.matmul(ptA[:W, :], sbk16_s[:, 0], s10_s[:], start=True, stop=False)
            tensor.wait_ge(a2t, 1)
            tensor.matmul(
                ptA[:W, :], sbk16_s[:, 0], sp_s[:, 0], start=False, stop=True
            ).then_inc(p2v, 1)
            for d in range(1, D1):
                if d == 2:
                    tensor.wait_ge(q_sbk16b, 16)
                    tensor.wait_ge(q_sbk32b, 16)
                if d == 4:
                    tensor.wait_ge(q_sbk32, 16)
                tensor.wait_ge(p2t, d)
                # s1 scatter: fp16 single-pass at step 1 (fp32 weights cannot
                # arrive that early), exact fp32 (2-pass, hidden under the
                # exp/ln window) for every later layer
                if d == 1:
                    tensor.matmul(
                        pts[d % 2][:W, :], sbk16_s[:, d], s1f_s[:, d % 2],
                        start=True, stop=False,
                    )
                elif d <= 3:
                    tensor.matmul(
                        pts[d % 2][:W, :], sbk32b_s[:, d - 2], s1w_s[:, d % 2],
                        start=True, stop=False,
                    )
                else:
                    tensor.matmul(
                        pts[d % 2][:W, :], sbk32_s[:, d - 4], s1w_s[:, d % 2],
                        start=True, stop=False,
                    )
                tensor.wait_ge(a2t, d + 1)
                tensor.matmul(
                    pts[d % 2][:W, :], sbk16_s[:, d], sp_s[:, d % 2],
                    start=False, stop=True,
                ).then_inc(p2v, 1)

    return nc


def _fast_path_ok(emissions, transitions, layer_ids, succ):
    if emissions.shape != (BATCH, C, N_LABELS):
        return False
    if transitions.shape != (N_LABELS, N_LABELS, C, C):
        return False
    if layer_ids.shape != (DEPTH, WIDTH) or succ.shape != (DEPTH, WIDTH, N_LABELS):
        return False
    if not np.array_equal(np.sort(layer_ids.reshape(-1)), np.arange(N_LABELS)):
        return False
    nz = succ != 0
    if nz.sum(axis=-1).max(initial=0) > 1:
        return False
    if nz[DEPTH - 1].any():
        return False
    if not ((succ == 0) | (succ == 1)).all():
        return False
    for d in range(DEPTH - 1):
        in_block = nz[d][:, layer_ids[d + 1]].sum()
        if in_block != nz[d].sum():
            return False
    return True


def _numpy_fallback(emissions, transitions, layer_ids, succ):
    messages = np.zeros_like(emissions)
    for d in range(layer_ids.shape[0]):
        layer = layer_ids[d]
        S = succ[d]
        x = emissions[:, :, layer] + messages[:, :, layer]          # [B,C,W]
        t = np.transpose(transitions[:, layer], (0, 2, 3, 1))       # [N,K,C,W]
        z = x[None, :, None, :, :] + t[:, None, :, :, :]            # [N,B,K,C,W]
        m = z.max(axis=3, keepdims=True)
        elem = np.squeeze(m, 3) + np.log(np.exp(z - m).sum(axis=3))
        messages = messages + np.einsum("nbkw,wn->bkn", elem, S)
    return messages


def kernel(emissions, transitions, layer_ids, succ):
    from concourse.bass_utils import run_bass_kernel_spmd

    emissions = np.ascontiguousarray(np.asarray(emissions, dtype=np.float32))
    transitions = np.ascontiguousarray(np.asarray(transitions, dtype=np.float32))
    layer_ids = np.asarray(layer_ids).astype(np.int64)
    succ = np.ascontiguousarray(np.asarray(succ, dtype=np.float32))

    if not _fast_path_ok(emissions, transitions, layer_ids, succ):
        return _numpy_fallback(emissions, transitions, layer_ids, succ)

    nz = succ != 0
    tgt = np.argmax(nz, axis=-1)  # [D, W]; 0 for empty rows (unused: S row = 0)

    # gathered 2x2 transition block per source node: tt_g[d, w, k, c]
    tt_g = transitions[tgt, layer_ids]
    # tt part of ett, layout [w, d, (c, k, b)] (b-replicated)
    ttp = np.broadcast_to(
        tt_g[:D1].transpose(1, 0, 3, 2)[..., None], (WIDTH, D1, C, C, B)
    ).reshape(WIDTH, D1, 32)

    # successor block restricted to the next layer's labels (entries are 0/1
    # so both casts are exact); fp32 copies only for the deep layers 4-8
    sbkf = np.empty((WIDTH, D1, WIDTH), dtype=np.float32)
    for d in range(D1):
        sbkf[:, d, :] = succ[d][:, layer_ids[d + 1]]
    sbk16 = sbkf.astype(np.float16)
    sbk32b = np.ascontiguousarray(sbkf[:, 2:4])
    sbk32 = np.ascontiguousarray(sbkf[:, 4:])

    em_sh = emissions.reshape(N_CORES, B, C, N_LABELS)
    in_maps = []
    for i in range(N_CORES):
        g = em_sh[i][:, :, layer_ids[:D1]]                  # [b, c, d, w]
        emp = np.broadcast_to(
            g.transpose(3, 2, 1, 0)[:, :, :, None, :], (WIDTH, D1, C, C, B)
        ).reshape(WIDTH, D1, 32)                            # [w, d, (c,k,b)]
        ett = np.ascontiguousarray(
            np.concatenate([emp, ttp], axis=2), dtype=np.float32
        )
        in_maps.append(
            {"ett": ett, "sbk32b": sbk32b, "sbk32": sbk32, "sbk16": sbk16}
        )

    if "nc" not in _BASS_CACHE:
        _BASS_CACHE["nc"] = _build_bass()
    res = run_bass_kernel_spmd(
        _BASS_CACHE["nc"], in_maps, core_ids=list(range(N_CORES))
    )

    out = np.zeros((BATCH, C, N_LABELS), dtype=np.float32)
    for i in range(N_CORES):
        m = res.results[i]["out"].reshape(WIDTH, D1, C, B)
        blk = np.zeros((B, C, N_LABELS), dtype=np.float32)
        blk[:, :, layer_ids[1:]] = m.transpose(3, 2, 1, 0)  # [b, c, j, w]
        out[i * B : (i + 1) * B] = blk
    return out
